# revision 1
# baseline (speedup 1.0000x reference)
"""Trainium2 Bass kernel for DeformConv2D (b=4, c=64, H=W=128, ks=3).

Sharding: 8 cores = (sample s = core//2) x (row-half = core%2). Each core
computes output rows [64*half, 64*half+64) of its sample.

Per-core dataflow:
  1. Load a 74-row halo slice of x (f32, CHW) into SBUF, zero-padded cols.
  2. Build XD in DRAM: bf16 [74*132 slots, 2 cols, 64 ch] -- HWC layout with
     duplicated column pairs so one 256B gather element = (2 cols x 64 ch).
  3. Offset conv on PE (9 taps, K=64 matmuls) -> offsets [18, 8192] in PSUM.
  4. PE-transpose offsets to [128 w, 64 t, 18]; DVE coordinate pipeline
     computes bilinear corner weights W4 and int16 gather indices (wrapped
     16-partition layout for dma_gather, staged via DRAM).
  5. dma_gather (36 calls: 9 kernel points x 4 t-chunks) from XD.
  6. DVE combine: xoff[w, t, n, ci] = sum_rc W4 * G   (TS + 3 STT per (t,n)).
  7. PE-transpose xoff -> [(n, ci), pix]; final conv = 5 accumulating
     matmuls (K=576 over (n, ci)) -> out [64 co, 128 pix] per row.
"""
import sys
import types
import numpy as np
import ml_dtypes

sys.path.insert(0, "/opt/trn_rl_repo")

BF16 = ml_dtypes.bfloat16
NCORES = 8
NR = 74          # XD rows (local): row k <-> abs padded row h0-4+k
WROW = 132       # XD row stride in 256B elements
NSLOT = NR * WROW


def _install_ntff_hook():
    if "antenv.axon_hooks" in sys.modules:
        return
    try:
        import antenv
        from trn_agent_boot.trn_boot import _ntff_profile_via_ctypes
    except Exception:
        return
    mod = types.ModuleType("antenv.axon_hooks")
    _hook = [None]
    mod.set_axon_ntff_profile_hook = lambda h: _hook.__setitem__(0, h)
    mod.get_axon_ntff_profile_hook = lambda: _hook[0]
    sys.modules["antenv.axon_hooks"] = mod
    antenv.axon_hooks = mod
    try:
        mod.set_axon_ntff_profile_hook(
            _ntff_profile_via_ctypes("/opt/axon/libaxon_pjrt.so"))
    except Exception:
        mod.set_axon_ntff_profile_hook(None)


_PROGRAM = None


def _build_program():
    global _PROGRAM
    if _PROGRAM is not None:
        return _PROGRAM
    from contextlib import ExitStack
    import concourse.bass as bass
    import concourse.tile as tile
    from concourse import mybir, bacc

    f32 = mybir.dt.float32
    bf16 = mybir.dt.bfloat16
    i16 = mybir.dt.int16
    i32 = mybir.dt.int32
    A = mybir.AluOpType

    nc = bacc.Bacc()
    # ---- I/O ----
    xg_p = nc.declare_dram_parameter("xg", [64, NR * 128], f32, isOutput=False)
    base2_p = nc.declare_dram_parameter("base2", [128, 64 * 18], f32, isOutput=False)
    xsc_p = nc.declare_dram_parameter("xsc", [128, 4], f32, isOutput=False)
    woff_p = nc.declare_dram_parameter("woff", [64, 9 * 18], f32, isOutput=False)
    wca_p = nc.declare_dram_parameter("wconv_a", [128, 256], bf16, isOutput=False)
    wcb_p = nc.declare_dram_parameter("wconv_b", [64, 64], bf16, isOutput=False)
    idf_p = nc.declare_dram_parameter("ident_f", [128, 128], f32, isOutput=False)
    idb_p = nc.declare_dram_parameter("ident_b", [128, 128], bf16, isOutput=False)
    out_p = nc.declare_dram_parameter("out", [64, 64 * 128], f32, isOutput=True)

    xd = nc.dram_tensor("xd", [NSLOT, 128], bf16)           # gather source
    gstage = nc.dram_tensor("gstage", [16, 9216], i16)      # idx staging

    with tile.TileContext(nc) as tc, ExitStack() as ctx:
        consts = ctx.enter_context(tc.tile_pool(name="consts", bufs=1))
        big = ctx.enter_context(tc.tile_pool(name="big", bufs=1))
        scratch = ctx.enter_context(tc.tile_pool(name="scratch", bufs=4))
        ps_seq = ctx.enter_context(tc.tile_pool(name="ps_seq", bufs=3, space="PSUM"))
        co_ctx = ExitStack()
        coords = co_ctx.enter_context(tc.tile_pool(name="coords", bufs=1))

        # ---------- load constants ----------
        base2 = consts.tile([128, 64 * 18], f32)
        nc.sync.dma_start(out=base2, in_=base2_p[:, :])
        xsc = consts.tile([128, 4], f32)
        nc.sync.dma_start(out=xsc, in_=xsc_p[:, :])
        woff = consts.tile([64, 9, 18], f32)
        nc.sync.dma_start(out=woff, in_=woff_p[:, :].rearrange("a (t c) -> a t c", t=9))
        wca = consts.tile([128, 256], bf16)
        nc.sync.dma_start(out=wca, in_=wca_p[:, :])
        wcb = consts.tile([64, 64], bf16)
        nc.sync.dma_start(out=wcb, in_=wcb_p[:, :])
        idf = consts.tile([128, 128], f32)
        nc.sync.dma_start(out=idf, in_=idf_p[:, :])
        idb = consts.tile([128, 128], bf16)
        nc.sync.dma_start(out=idb, in_=idb_p[:, :])

        # ---------- phase A: x load + XD build ----------
        ab_ctx = ExitStack()
        abp = ab_ctx.enter_context(tc.tile_pool(name="abp", bufs=1))
        xsb = abp.tile([64, NR, WROW], f32, name="xsb")  # padded CHW slab
        nc.vector.memset(xsb, 0.0)
        nc.sync.dma_start(
            out=xsb[:, :, 1:129],
            in_=xg_p[:, :].rearrange("c (r w) -> c r w", r=NR))

        # zero-fill whole XD (borders + unwritten rows)
        zb = abp.tile([128, NSLOT // 8], bf16, name='zb')       # 9768/8 = 1221 per part
        nc.vector.memset(zb, 0.0)
        for c8 in range(8):
            nc.sync.dma_start(
                out=bass.AP(tensor=xd, offset=c8 * (NSLOT // 8),
                            ap=[[NSLOT, 128], [1, NSLOT // 8]]),
                in_=zb)

        # transpose x rows -> bf16 HWC, write slot0/slot1 into XD
        for blk in range(10):                        # 8 rows per block; 74 rows
            rows = min(8, NR - blk * 8)
            pst = ps_seq.tile([128, 512], f32, tag="seq")
            for j in range(rows):
                k = blk * 8 + j
                nc.tensor.transpose(
                    pst[:, j * 64:(j + 1) * 64], xsb[:, k, 1:129],
                    idf[0:64, 0:64])
            xrb = scratch.tile([128, 8, 64], bf16, tag="xrow")
            nc.any.tensor_copy(
                xrb[:, 0:rows, :],
                pst[:, 0:rows * 64].rearrange("p (r c) -> p r c", r=rows))
            # slot0: XD[k, w, 0, :] = x[., k, w] for w=1..128 (padded col idx)
            nc.sync.dma_start(
                out=bass.AP(tensor=xd,
                            offset=(blk * 8 * WROW + 1) * 128 + 0,
                            ap=[[128, 128], [WROW * 128, rows], [1, 64]]),
                in_=xrb[:, 0:rows, :])
            # slot1: XD[k, w-1, 1, :] = x[., k, w]
            nc.sync.dma_start(
                out=bass.AP(tensor=xd,
                            offset=(blk * 8 * WROW + 0) * 128 + 64,
                            ap=[[128, 128], [WROW * 128, rows], [1, 64]]),
                in_=xrb[:, 0:rows, :])

        # ---------- phase B: offset conv ----------
        off_sb = abp.tile([18, 64, 128], f32, name='off_sb')
        for tb in range(16):                         # 4 output rows per tile
            psc = ps_seq.tile([18, 512], f32, tag="seq")
            for dy in range(3):
                for dx in range(3):
                    tap = dy * 3 + dx
                    nc.tensor.matmul(
                        psc[:, :],
                        woff[:, tap, :],
                        bass.AP(tensor=xsb.tensor,
                                offset=xsb.offset + (tb * 4 + dy + 4) * WROW + dx,
                                ap=[xsb.ap[0], [WROW, 4], [1, 128]]),
                        start=(tap == 0), stop=(tap == 8))
            nc.any.tensor_copy(
                off_sb[:, tb * 4:tb * 4 + 4, :],
                psc[:, :].rearrange("p (r w) -> p r w", r=4))

        # transpose offsets -> offt [128 w, 64 t, 18]
        offt = coords.tile([128, 64, 18], f32)
        for b in range(4):
            pst = ps_seq.tile([128, 288], f32, tag="seq")
            for j in range(16):
                t = b * 16 + j
                nc.tensor.transpose(
                    pst[:, j * 18:(j + 1) * 18],
                    off_sb[:, t, :], idf[0:18, 0:18])
            nc.any.tensor_copy(
                offt[:, b * 16:(b + 1) * 16, :],
                pst[:, :].rearrange("p (t c) -> p t c", t=16))
        ab_ctx.close()

        # ---------- phase C: coordinates ----------
        def cT(shape, tag):
            return coords.tile(shape, f32, tag=tag, name=tag)

        P = cT([128, 64, 18], "P")
        nc.vector.tensor_tensor(
            P, offt, base2.rearrange("p (t c) -> p t c", t=64), A.add)
        q_i = coords.tile([128, 64, 18], i32, tag="cs", name="qi", bufs=4)
        nc.vector.tensor_copy(q_i, P)
        Qf0 = coords.tile([128, 64, 18], f32, tag="cs", name="qf0", bufs=4)
        nc.vector.tensor_copy(Qf0, q_i)
        GT = coords.tile([128, 64, 18], f32, tag="cs", name="gt", bufs=4)
        nc.vector.tensor_tensor(GT, Qf0, P, A.is_gt)
        Qf = cT([128, 64, 18], "qf")
        nc.vector.tensor_tensor(Qf, Qf0, GT, A.subtract)
        FR = coords.tile([128, 64, 18], f32, tag="cs", name="fr", bufs=4)
        nc.vector.tensor_tensor(FR, P, Qf, A.subtract)
        INR = coords.tile([128, 64, 18], f32, tag="cs", name="inr", bufs=4)
        # x half: per-core bounds via scalar APs; y half: immediates
        nc.vector.tensor_scalar(INR[:, :, 0:9], P[:, :, 0:9],
                                xsc[:, 0:1], None, A.is_ge)
        nc.vector.tensor_scalar(INR[:, :, 9:18], P[:, :, 9:18],
                                9.0, None, A.is_ge)
        INH = coords.tile([128, 64, 18], f32, tag="cs", name="inh", bufs=4)
        nc.vector.tensor_scalar(INH[:, :, 0:9], P[:, :, 0:9],
                                xsc[:, 1:2], None, A.is_le)
        nc.vector.tensor_scalar(INH[:, :, 9:18], P[:, :, 9:18],
                                136.0, None, A.is_le)
        nc.vector.tensor_tensor(INR, INR, INH, A.mult)
        FRV = cT([128, 64, 18], "frv")
        nc.vector.tensor_tensor(FRV, FR, INR, A.mult)
        ALT = cT([128, 64, 18], "alt")
        nc.vector.tensor_scalar(ALT, FRV, -1.0, 1.0, A.mult, A.add)
        QC = cT([128, 64, 18], "qc")
        nc.vector.tensor_scalar(QC[:, :, 0:9], Qf[:, :, 0:9],
                                xsc[:, 2:3], xsc[:, 3:4], A.max, A.min)
        nc.vector.tensor_scalar(QC[:, :, 9:18], Qf[:, :, 9:18],
                                8.0, 137.0, A.max, A.min)
        # gather linear indices (f32 exact ints)
        LINF = cT([128, 64, 9], "linf")
        nc.vector.tensor_scalar(LINF, QC[:, :, 0:9], 132.0, -536.0, A.mult, A.add)
        nc.vector.tensor_tensor(LINF, LINF, QC[:, :, 9:18], A.add)
        LIN2 = coords.tile([128, 9, 64, 2], f32, tag="lin2", name="lin2")
        linf_T = bass.AP(tensor=LINF.tensor, offset=LINF.offset,
                         ap=[LINF.ap[0], [1, 9], [9, 64]])
        nc.vector.tensor_copy(LIN2[:, :, :, 0], linf_T)
        nc.vector.tensor_scalar(LIN2[:, :, :, 1], linf_T, 132.0, None, A.add)
        gidx_pre = coords.tile([128, 9, 64, 2], i16, tag="gpre", name="gpre")
        nc.vector.tensor_copy(gidx_pre, LIN2)
        # corner weight products [128, 64t, 9n, 4rc]
        W4 = consts.tile([128, 64, 9, 4], f32, tag="w4", name="w4")
        nc.vector.tensor_tensor(W4[:, :, :, 0], ALT[:, :, 0:9], ALT[:, :, 9:18], A.mult)
        nc.vector.tensor_tensor(W4[:, :, :, 1], ALT[:, :, 0:9], FRV[:, :, 9:18], A.mult)
        nc.vector.tensor_tensor(W4[:, :, :, 2], FRV[:, :, 0:9], ALT[:, :, 9:18], A.mult)
        nc.vector.tensor_tensor(W4[:, :, :, 3], FRV[:, :, 0:9], FRV[:, :, 9:18], A.mult)

        # ---------- idx relayout to wrapped-16 (via DRAM staging) ----------
        # gstage[pl, n, tc, tt, r, ph] = gidx_pre[ph*16+pl, tc*16+tt, n, r]
        for ph in range(8):
            sl = gidx_pre[ph * 16:ph * 16 + 16]
            nc.sync.dma_start(
                out=bass.AP(tensor=gstage, offset=ph * 1152,
                            ap=[[9216, 16], [1, 1152]]),
                in_=bass.AP(tensor=sl.tensor, offset=sl.offset,
                            ap=[sl.ap[0], [1, 1152]]))
        sg = consts.tile([128, 8, 1152], i16, name="sg")
        nc.gpsimd.dma_start(
            out=sg,
            in_=bass.AP(tensor=gstage, offset=0,
                        ap=[[0, 8], [9216, 16], [1, 9216]]))
        gidx = consts.tile([128, 9, 4, 256], i16)
        # ph-interleave on DVE: gidx[p, j2*8+ph] = sg[p, ph, j2]
        nc.vector.tensor_copy(
            bass.AP(tensor=gidx.tensor, offset=gidx.offset,
                    ap=[gidx.ap[0], [1, 8], [8, 1152]]),
            sg)

        # pre-drain gather deps onto the Pool engine (the DMA-gather ISA
        # struct supports very few semaphore waits)
        j1 = scratch.tile([16, 8], bf16, tag="join", name="j1")
        nc.sync.dma_start(out=j1[0:1, 0:8], in_=xd[0:1, 0:8])
        j2 = scratch.tile([16, 8], i16, tag="join2", name="j2")
        j3 = scratch.tile([16, 8], bf16, tag="join3", name="j3")
        nc.gpsimd.tensor_copy(j2[0:16, 0:4], gidx[0:16, 0, 0, 0:4])
        nc.gpsimd.tensor_copy(j3[0:1, 0:4], j1[0:1, 0:4])

        # ---------- phase D: gather + combine + final conv ----------
        co_ctx.close()
        ps_x = ctx.enter_context(tc.tile_pool(name="ps_x", bufs=2, space="PSUM"))
        ps_o = ctx.enter_context(tc.tile_pool(name="ps_o", bufs=2, space="PSUM"))
        gpool = ctx.enter_context(tc.tile_pool(name="gpool", bufs=3))
        xpool = ctx.enter_context(tc.tile_pool(name="xpool", bufs=2))
        tpool = ctx.enter_context(tc.tile_pool(name="tpool", bufs=8))
        rpool = ctx.enter_context(tc.tile_pool(name="rpool", bufs=3))
        for tcn in range(4):                         # t-chunks of 16 rows
            outb = big.tile([64, 16, 128], f32, tag="outb", bufs=2, name="outb")
            xoff = xpool.tile([128, 16, 9, 64], bf16, tag="xoff", name="xoff")
            gs = []
            for n in range(9):
                g = gpool.tile([128, 16, 2, 2, 64], bf16, tag="g")
                nc.gpsimd.dma_gather(
                    out_ap=g.rearrange("p a b c d -> p (a b) (c d)"),
                    in_ap=xd[:, :],
                    idxs_ap=gidx[:, n, tcn, :],
                    num_idxs=4096,
                    num_idxs_reg=4096,
                    elem_size=128,
                    single_packet=False,
                )
                gs.append(g)
            for n in range(9):
                g = gs[n]
                for tt in range(16):
                    t = tcn * 16 + tt
                    tmp = tpool.tile([128, 64], bf16, tag="tmp")
                    nc.vector.tensor_scalar(
                        tmp, g[:, tt, 0, 0, :], W4[:, t, n, 0:1], None, A.mult)
                    tmp2 = tpool.tile([128, 64], bf16, tag="tmp")
                    nc.vector.scalar_tensor_tensor(
                        tmp2, g[:, tt, 0, 1, :], W4[:, t, n, 1:2], tmp,
                        A.mult, A.add)
                    tmp3 = tpool.tile([128, 64], bf16, tag="tmp")
                    nc.vector.scalar_tensor_tensor(
                        tmp3, g[:, tt, 1, 0, :], W4[:, t, n, 2:3], tmp2,
                        A.mult, A.add)
                    nc.vector.scalar_tensor_tensor(
                        xoff[:, tt, n, :], g[:, tt, 1, 1, :], W4[:, t, n, 3:4],
                        tmp3, A.mult, A.add)
            # transpose xoff per row, final conv
            for tt in range(16):
                t = tcn * 16 + tt
                pso = ps_o.tile([64, 128], f32, tag="o")
                for jc in range(4):
                    psx = ps_x.tile([128, 128], bf16, tag="x")
                    nc.tensor.transpose(
                        psx,
                        xoff[:, tt, 2 * jc:2 * jc + 2, :].rearrange(
                            "p a b -> p (a b)"),
                        idb)
                    rhs = rpool.tile([128, 128], bf16, tag="r")
                    nc.any.tensor_copy(rhs, psx)
                    nc.tensor.matmul(pso, wca[:, jc * 64:(jc + 1) * 64], rhs,
                     start=(jc == 0), stop=False)
                psx4 = ps_x.tile([128, 128], bf16, tag="x")
                nc.tensor.transpose(
                    psx4[0:64, :], xoff[:, tt, 8, :], idb)
                rhs4 = rpool.tile([64, 128], bf16, tag="r4")
                nc.any.tensor_copy(rhs4, psx4[0:64, :])
                nc.tensor.matmul(pso, wcb, rhs4, start=False, stop=True)
                nc.any.tensor_copy(outb[:, tt, :], pso)

            nc.sync.dma_start(
                out=out_p[:, tcn * 2048:(tcn + 1) * 2048],
                in_=outb.rearrange("c t w -> c (t w)"))

    nc.finalize()
    _PROGRAM = nc
    return nc


def _host_consts(W_off, b_off, W_conv):
    idxr = np.concatenate([np.arange(0, 18, 2), np.arange(1, 18, 2)])
    W_off_r = W_off[idxr]            # (18, 64, 3, 3)
    b_off_r = b_off[idxr]            # (18,)
    woff = np.ascontiguousarray(
        W_off_r.transpose(2, 3, 1, 0).reshape(9, 64, 18).transpose(1, 0, 2)
    ).reshape(64, 9 * 18).astype(np.float32)
    # base2 [128 w, 64 t, 18]
    nidx = np.arange(9)
    pnx = (nidx // 3) - 1
    pny = (nidx % 3) - 1
    tt = np.arange(64)
    ww = np.arange(128)
    base2 = np.zeros((128, 64, 18), np.float32)
    base2[:, :, 0:9] = tt[None, :, None] + 9 + pnx[None, None, :] + \
        b_off_r[None, None, 0:9]
    base2[:, :, 9:18] = ww[:, None, None] + 9 + pny[None, None, :] + \
        b_off_r[None, None, 9:18]
    base2 = base2.reshape(128, 64 * 18)
    # final conv weights
    Wmat = W_conv.reshape(64, 64, 9).transpose(0, 2, 1)   # (co, n, ci)
    wca = np.zeros((128, 256), np.float32)
    for jc in range(4):
        for dn in range(2):
            # K row = dn*64+ci ; col block jc : [K, co]
            wca[dn * 64:(dn + 1) * 64, jc * 64:(jc + 1) * 64] = \
                Wmat[:, 2 * jc + dn, :].T
    wcb = np.ascontiguousarray(Wmat[:, 8, :].T)           # (ci, co)
    return {
        "woff": woff,
        "base2": base2,
        "wconv_a": wca.astype(BF16),
        "wconv_b": wcb.astype(BF16),
        "ident_f": np.eye(128, dtype=np.float32),
        "ident_b": np.eye(128, dtype=np.float32).astype(BF16),
    }


def _per_core_inputs(x, consts, s, half):
    h0 = 64 * half
    xs = x[s]                                    # (64, 128, 128)
    xgs = np.zeros((64, NR, 128), np.float32)
    lo = h0 - 5                                  # unpadded row of xg row 0
    for k in range(NR):
        r = lo + k
        if 0 <= r < 128:
            xgs[:, k, :] = xs[:, r, :]
    xsc = np.zeros((128, 4), np.float32)
    xsc[:, 0] = 9 - h0                           # mask lo
    xsc[:, 1] = 136 - h0                         # mask hi
    xsc[:, 2] = 8 - min(h0, 2)                   # clip lo (tightened)
    xsc[:, 3] = min(129, h0 + 69) - h0 + 8       # clip hi (tightened)
    return {
        "xg": xgs.reshape(64, NR * 128),
        "xsc": xsc,
        **consts,
    }


def kernel(x, W_off, b_off, W_conv):
    _install_ntff_hook()
    # the bass kernel must run on the axon trn2 backend; undo any cpu pin
    # (e.g. a harness that set JAX_PLATFORMS=cpu for the reference)
    import os
    if os.environ.get("JAX_PLATFORMS", "") == "cpu":
        try:
            import jax
            jax.config.update("jax_platforms", None)
            os.environ.pop("JAX_PLATFORMS", None)
        except Exception:
            pass
    x = np.asarray(x, np.float32)
    W_off = np.asarray(W_off, np.float32)
    b_off = np.asarray(b_off, np.float32)
    W_conv = np.asarray(W_conv, np.float32)

    from concourse.bass_utils import run_bass_kernel_spmd
    nc = _build_program()
    consts = _host_consts(W_off, b_off, W_conv)
    in_maps = [
        _per_core_inputs(x, consts, core // 2, core % 2) for core in range(NCORES)
    ]
    res = run_bass_kernel_spmd(nc, in_maps, list(range(NCORES)))
    out = np.empty((4, 64, 128, 128), np.float32)
    for core in range(NCORES):
        s, half = core // 2, core % 2
        out[s, :, 64 * half:64 * half + 64, :] = \
            res.results[core]["out"].reshape(64, 64, 128)
    return out



# revision 2
# speedup vs baseline: 1.0174x; 1.0174x over previous
"""Trainium2 Bass kernel for DeformConv2D (b=4, c=64, H=W=128, ks=3). v3.

Sharding: 8 cores = (sample s = core//2) x (row-half = core%2). Each core
computes output rows [64*half, 64*half+64) of its sample.

v5 changes vs v3:
  - Offset conv K-packs taps (dy,0)+(dy,1) into K=128 matmuls using a
    128-partition slab whose upper 64 partitions hold x shifted one column
    (6 matmuls per 4-row tile, full PE width).
  - xsb memset narrowed to the pad columns only (the full-slab memset was
    ~28us of DVE ahead of the first chunk's coordinate pipeline).

v3 changes vs v2:
  - XD (gather source, column-major 512B corner-pair layout) is built
    host-side with numpy (pure layout transform + bf16 cast) and passed as
    an input parameter -- removes ~150us of PE transposes from the kernel
    prologue.
  - Offset conv -> coords -> idx staging are chunked per 16 output rows so
    the first gather issues after ~1/4 of the prologue.
  - Final conv batches 4 output rows per PSUM accumulation (20 matmuls per
    chunk instead of 80).
"""
import sys
import types
import numpy as np
import ml_dtypes

sys.path.insert(0, "/opt/trn_rl_repo")

BF16 = ml_dtypes.bfloat16
NCORES = 8
NR = 74          # data rows (local): row k <-> abs padded row h0-4+k
NRC = 76         # k extent per column in XD (2 zero pad rows)
NC2 = 130        # col-pair slots c = QCy-8 in [0, 129]
NSLOT2 = NC2 * NRC          # 9880 referenced slots
XD2T = 9888                 # declared slots (pad to /8 and AP bounds)


def _install_ntff_hook():
    if "antenv.axon_hooks" in sys.modules:
        return
    try:
        import antenv
        from trn_agent_boot.trn_boot import _ntff_profile_via_ctypes
    except Exception:
        return
    mod = types.ModuleType("antenv.axon_hooks")
    _hook = [None]
    mod.set_axon_ntff_profile_hook = lambda h: _hook.__setitem__(0, h)
    mod.get_axon_ntff_profile_hook = lambda: _hook[0]
    sys.modules["antenv.axon_hooks"] = mod
    antenv.axon_hooks = mod
    try:
        mod.set_axon_ntff_profile_hook(
            _ntff_profile_via_ctypes("/opt/axon/libaxon_pjrt.so"))
    except Exception:
        mod.set_axon_ntff_profile_hook(None)


_PROGRAM = None


def _build_program():
    global _PROGRAM
    if _PROGRAM is not None:
        return _PROGRAM
    from contextlib import ExitStack
    import concourse.bass as bass
    import concourse.tile as tile
    from concourse import mybir, bacc

    f32 = mybir.dt.float32
    bf16 = mybir.dt.bfloat16
    i16 = mybir.dt.int16
    i32 = mybir.dt.int32
    A = mybir.AluOpType

    nc = bacc.Bacc()
    # ---- I/O ----
    xg_p = nc.declare_dram_parameter("xg", [64, NR * 128], f32, isOutput=False)
    xd_p = nc.declare_dram_parameter("xd", [XD2T, 128], bf16, isOutput=False)
    base2_p = nc.declare_dram_parameter("base2", [128, 64 * 18], f32, isOutput=False)
    xsc_p = nc.declare_dram_parameter("xsc", [128, 4], f32, isOutput=False)
    woff_p = nc.declare_dram_parameter("woff", [64, 9 * 18], f32, isOutput=False)
    woffa_p = nc.declare_dram_parameter("woffa", [128, 54], f32, isOutput=False)
    wca_p = nc.declare_dram_parameter("wconv_a", [128, 256], bf16, isOutput=False)
    wcb_p = nc.declare_dram_parameter("wconv_b", [64, 64], bf16, isOutput=False)
    idf_p = nc.declare_dram_parameter("ident_f", [128, 128], f32, isOutput=False)
    idb_p = nc.declare_dram_parameter("ident_b", [128, 128], bf16, isOutput=False)
    out_p = nc.declare_dram_parameter("out", [64, 64 * 128], f32, isOutput=True)

    gstage = nc.dram_tensor("gstage", [16, 4608], i16)      # idx staging

    xd_ap0 = xd_p[:, :]
    xd_gather = bass.AP(tensor=xd_ap0.tensor, offset=0,
                        ap=[[128, NSLOT2], [1, 256]])

    with tile.TileContext(nc) as tc, ExitStack() as ctx:
        consts = ctx.enter_context(tc.tile_pool(name="consts", bufs=1))
        big = ctx.enter_context(tc.tile_pool(name="big", bufs=1))
        scratch = ctx.enter_context(tc.tile_pool(name="scratch", bufs=4))
        ps_seq = ctx.enter_context(tc.tile_pool(name="ps_seq", bufs=3, space="PSUM"))
        coords = ctx.enter_context(tc.tile_pool(name="coords", bufs=2))
        abp = ctx.enter_context(tc.tile_pool(name="abp", bufs=1))

        # ---------- load constants ----------
        base2 = consts.tile([128, 64 * 18], f32)
        nc.sync.dma_start(out=base2, in_=base2_p[:, :])
        xsc = consts.tile([128, 4], f32)
        nc.sync.dma_start(out=xsc, in_=xsc_p[:, :])
        woff = consts.tile([64, 9, 18], f32)
        nc.sync.dma_start(out=woff, in_=woff_p[:, :].rearrange("a (t c) -> a t c", t=9))
        woffa = consts.tile([128, 3, 18], f32)
        nc.sync.dma_start(out=woffa, in_=woffa_p[:, :].rearrange("a (t c) -> a t c", t=3))
        wca = consts.tile([128, 256], bf16)
        nc.sync.dma_start(out=wca, in_=wca_p[:, :])
        wcb = consts.tile([64, 64], bf16)
        nc.sync.dma_start(out=wcb, in_=wcb_p[:, :])
        idf = consts.tile([128, 128], f32)
        nc.sync.dma_start(out=idf, in_=idf_p[:, :])
        idb = consts.tile([128, 128], bf16)
        nc.sync.dma_start(out=idb, in_=idb_p[:, :])

        # ---------- x slab (offset conv input) ----------
        # partitions 0-63: x at padded cols 1..128; partitions 64-127: the
        # same data shifted one column left (so a K=128 matmul covers taps
        # dx=0 and dx=1 at once); memset only the pad columns
        xsb = abp.tile([128, NR, 132], f32, name="xsb")
        nc.vector.memset(xsb[:, :, 0:1], 0.0)
        nc.vector.memset(xsb[0:64, :, 129:132], 0.0)
        nc.vector.memset(xsb[64:128, :, 128:132], 0.0)
        nc.sync.dma_start(
            out=xsb[0:64, :, 1:129],
            in_=xg_p[:, :].rearrange("c (r w) -> c r w", r=NR))
        nc.sync.dma_start(
            out=xsb[64:128, :, 0:128],
            in_=xg_p[:, :].rearrange("c (r w) -> c r w", r=NR))

        # ---------- prologue per t-chunk: offset conv + coords + idxs ----
        base2v = base2.rearrange("p (t c) -> p t c", t=64)
        W4s = []
        gidxs = []
        for tcn in range(4):
            # offset conv rows [16*tcn, 16*tcn+16)
            off_sb = coords.tile([18, 16, 128], f32, tag="off", name="off")
            for tb4 in range(4):
                tb = tcn * 4 + tb4
                psc = ps_seq.tile([18, 512], f32, tag="seq")
                for dy in range(3):
                    # taps (dy, 0) + (dy, 1) fused over K=128
                    nc.tensor.matmul(
                        psc[:, :],
                        woffa[:, dy, :],
                        bass.AP(tensor=xsb.tensor,
                                offset=xsb.offset + (tb * 4 + dy + 4) * 132,
                                ap=[xsb.ap[0], [132, 4], [1, 128]]),
                        start=(dy == 0), stop=False)
                for dy in range(3):
                    # tap (dy, 2), K=64
                    nc.tensor.matmul(
                        psc[:, :],
                        woff[:, dy * 3 + 2, :],
                        bass.AP(tensor=xsb.tensor,
                                offset=xsb.offset + (tb * 4 + dy + 4) * 132 + 2,
                                ap=[[xsb.ap[0][0], 64], [132, 4], [1, 128]]),
                        start=False, stop=(dy == 2))
                nc.any.tensor_copy(
                    off_sb[:, tb4 * 4:tb4 * 4 + 4, :],
                    psc[:, :].rearrange("p (r w) -> p r w", r=4))

            # transpose offsets -> offt [128 w, 16 t, 18]
            offt = coords.tile([128, 16, 18], f32, tag="offt", name="offt")
            pst = ps_seq.tile([128, 288], f32, tag="seq")
            for j in range(16):
                nc.tensor.transpose(
                    pst[:, j * 18:(j + 1) * 18],
                    off_sb[:, j, :], idf[0:18, 0:18])
            nc.any.tensor_copy(
                offt, pst[:, :].rearrange("p (t c) -> p t c", t=16))

            # coordinates for this chunk
            def cT(tag):
                return coords.tile([128, 16, 18], f32, tag=tag, name=tag,
                                   bufs=2)

            P = cT("P")
            nc.vector.tensor_tensor(
                P, offt, base2v[:, tcn * 16:(tcn + 1) * 16, :], A.add)
            q_i = coords.tile([128, 16, 18], i32, tag="qi", name="qi", bufs=2)
            nc.vector.tensor_copy(q_i, P)
            Qf0 = cT("qf0")
            nc.vector.tensor_copy(Qf0, q_i)
            GT = cT("gt")
            nc.vector.tensor_tensor(GT, Qf0, P, A.is_gt)
            Qf = cT("qf")
            nc.vector.tensor_tensor(Qf, Qf0, GT, A.subtract)
            FR = cT("fr")
            nc.vector.tensor_tensor(FR, P, Qf, A.subtract)
            INR = cT("inr")
            nc.vector.tensor_scalar(INR[:, :, 0:9], P[:, :, 0:9],
                                    xsc[:, 0:1], None, A.is_ge)
            nc.vector.tensor_scalar(INR[:, :, 9:18], P[:, :, 9:18],
                                    9.0, None, A.is_ge)
            INH = cT("inh")
            nc.vector.tensor_scalar(INH[:, :, 0:9], P[:, :, 0:9],
                                    xsc[:, 1:2], None, A.is_le)
            nc.vector.tensor_scalar(INH[:, :, 9:18], P[:, :, 9:18],
                                    136.0, None, A.is_le)
            nc.vector.tensor_tensor(INR, INR, INH, A.mult)
            FRV = cT("frv")
            nc.vector.tensor_tensor(FRV, FR, INR, A.mult)
            ALT = cT("alt")
            nc.vector.tensor_scalar(ALT, FRV, -1.0, 1.0, A.mult, A.add)
            QC = cT("qc")
            nc.vector.tensor_scalar(QC[:, :, 0:9], Qf[:, :, 0:9],
                                    xsc[:, 2:3], xsc[:, 3:4], A.max, A.min)
            nc.vector.tensor_scalar(QC[:, :, 9:18], Qf[:, :, 9:18],
                                    8.0, 137.0, A.max, A.min)
            # gather idx: slot = 76*(QCy-8) + (QCx-4) = 76*QCy + QCx - 612
            LINF = coords.tile([128, 16, 9], f32, tag="linf", name="linf",
                               bufs=2)
            nc.vector.tensor_scalar(LINF, QC[:, :, 9:18], float(NRC), -612.0,
                                    A.mult, A.add)
            nc.vector.tensor_tensor(LINF, LINF, QC[:, :, 0:9], A.add)
            gidx_pre = coords.tile([128, 9, 16], i16, tag="gpre", name="gpre",
                                   bufs=2)
            linf_T = bass.AP(tensor=LINF.tensor, offset=LINF.offset,
                             ap=[LINF.ap[0], [1, 9], [9, 16]])
            nc.vector.tensor_copy(gidx_pre, linf_T)
            # corner weights [128, 16t, 9n, 4rc]; rc order matches the 512B
            # gather element: [(k,c), (k,c+1), (k+1,c), (k+1,c+1)]
            W4 = coords.tile([128, 16, 9, 4], f32, tag="w4", name="w4",
                             bufs=2)
            nc.vector.tensor_tensor(W4[:, :, :, 0], ALT[:, :, 0:9],
                                    ALT[:, :, 9:18], A.mult)
            nc.vector.tensor_tensor(W4[:, :, :, 1], ALT[:, :, 0:9],
                                    FRV[:, :, 9:18], A.mult)
            nc.vector.tensor_tensor(W4[:, :, :, 2], FRV[:, :, 0:9],
                                    ALT[:, :, 9:18], A.mult)
            nc.vector.tensor_tensor(W4[:, :, :, 3], FRV[:, :, 0:9],
                                    FRV[:, :, 9:18], A.mult)
            W4s.append(W4)

            # idx relayout to wrapped-16 via DRAM staging:
            # consumption j = col*16 + pl = t*128 + w  ->  col = t*8 + ph
            for ph in range(8):
                sl = gidx_pre[ph * 16:ph * 16 + 16]
                nc.sync.dma_start(
                    out=bass.AP(tensor=gstage, offset=tcn * 1152 + ph * 144,
                                ap=[[4608, 16], [1, 144]]),
                    in_=bass.AP(tensor=sl.tensor, offset=sl.offset,
                                ap=[sl.ap[0], [1, 144]]))
            sg = coords.tile([128, 8, 144], i16, tag="sg", name="sg", bufs=2)
            nc.gpsimd.dma_start(
                out=sg,
                in_=bass.AP(tensor=gstage, offset=tcn * 1152,
                            ap=[[0, 8], [4608, 16], [1, 1152]]))
            gidx = coords.tile([128, 9, 128], i16, tag="gidx", name="gidx",
                               bufs=2)
            nc.vector.tensor_copy(
                bass.AP(tensor=gidx.tensor, offset=gidx.offset,
                        ap=[gidx.ap[0], [1, 8], [8, 144]]),
                sg)
            gidxs.append(gidx)

        # ---------- phase D: gather + combine + final conv ----------
        ps_x = ctx.enter_context(tc.tile_pool(name="ps_x", bufs=2, space="PSUM"))
        ps_o = ctx.enter_context(tc.tile_pool(name="ps_o", bufs=2, space="PSUM"))
        gpool = ctx.enter_context(tc.tile_pool(name="gpool", bufs=3))
        xpool = ctx.enter_context(tc.tile_pool(name="xpool", bufs=2))
        tpool = ctx.enter_context(tc.tile_pool(name="tpool", bufs=8))
        rpool = ctx.enter_context(tc.tile_pool(name="rpool", bufs=2))

        for tcn in range(4):                         # t-chunks of 16 rows
            W4 = W4s[tcn]
            gidx = gidxs[tcn]

            def w4b(n, rc, W4=W4):
                # W4[:, :, n, rc] broadcast over ci (stride 0)
                return bass.AP(
                    tensor=W4.tensor,
                    offset=W4.offset + n * 4 + rc,
                    ap=[W4.ap[0], [36, 16], [0, 64]])

            # pre-drain: put the idx dependency on Pool's own stream
            j2 = scratch.tile([16, 8], i16, tag="join2", name="j2")
            nc.gpsimd.tensor_copy(j2[0:16, 0:4], gidx[0:16, 0, 0:4])

            outb = big.tile([64, 16, 128], f32, tag="outb", bufs=2, name="outb")
            xoff = xpool.tile([128, 16, 9, 64], bf16, tag="xoff", name="xoff")
            for n in range(9):
                g = gpool.tile([128, 16, 4, 64], bf16, tag="g")
                nc.gpsimd.dma_gather(
                    out_ap=g.rearrange("p a b c -> p a (b c)"),
                    in_ap=xd_gather,
                    idxs_ap=gidx[:, n, :],
                    num_idxs=2048,
                    num_idxs_reg=2048,
                    elem_size=256,
                    elem_step=128,
                    single_packet=False,
                )
                # bilinear combine: xoff[:, :, n, :] = sum_rc W4 * g[..rc..]
                m0 = tpool.tile([128, 16, 64], bf16, tag="tmp")
                m1 = tpool.tile([128, 16, 64], bf16, tag="tmp")
                m2 = tpool.tile([128, 16, 64], bf16, tag="tmp")
                m3 = tpool.tile([128, 16, 64], bf16, tag="tmp")
                nc.vector.tensor_tensor(m0, g[:, :, 0, :], w4b(n, 0), A.mult)
                nc.vector.tensor_tensor(m1, g[:, :, 1, :], w4b(n, 1), A.mult)
                nc.vector.tensor_tensor(m2, g[:, :, 2, :], w4b(n, 2), A.mult)
                nc.vector.tensor_tensor(m3, g[:, :, 3, :], w4b(n, 3), A.mult)
                nc.vector.tensor_tensor(m0, m0, m1, A.add)
                nc.vector.tensor_tensor(m2, m2, m3, A.add)
                nc.vector.tensor_tensor(xoff[:, :, n, :], m0, m2, A.add)

            # final conv: 4 output rows batched per PSUM accumulation
            for tq in range(4):
                pso = ps_o.tile([64, 4, 128], f32, tag="o")
                rhs = rpool.tile([128, 4, 128], bf16, tag="r")
                rhs4 = rpool.tile([64, 4, 128], bf16, tag="r4")
                for jc in range(4):
                    for r in range(4):
                        tt = tq * 4 + r
                        psx = ps_x.tile([128, 128], bf16, tag="x")
                        nc.tensor.transpose(
                            psx,
                            xoff[:, tt, 2 * jc:2 * jc + 2, :].rearrange(
                                "p a b -> p (a b)"),
                            idb)
                        nc.any.tensor_copy(rhs[:, r, :], psx)
                    nc.tensor.matmul(
                        pso.rearrange("p a b -> p (a b)"),
                        wca[:, jc * 64:(jc + 1) * 64],
                        rhs.rearrange("p a b -> p (a b)"),
                        start=(jc == 0), stop=False)
                for r in range(4):
                    tt = tq * 4 + r
                    psx4 = ps_x.tile([128, 128], bf16, tag="x")
                    nc.tensor.transpose(
                        psx4[0:64, :], xoff[:, tt, 8, :], idb)
                    nc.any.tensor_copy(rhs4[:, r, :], psx4[0:64, :])
                nc.tensor.matmul(
                    pso.rearrange("p a b -> p (a b)"),
                    wcb,
                    rhs4.rearrange("p a b -> p (a b)"),
                    start=False, stop=True)
                nc.any.tensor_copy(outb[:, tq * 4:(tq + 1) * 4, :], pso)

            nc.sync.dma_start(
                out=out_p[:, tcn * 2048:(tcn + 1) * 2048],
                in_=outb.rearrange("c t w -> c (t w)"))

    nc.finalize()
    _PROGRAM = nc
    return nc


def _host_consts(W_off, b_off, W_conv):
    idxr = np.concatenate([np.arange(0, 18, 2), np.arange(1, 18, 2)])
    W_off_r = W_off[idxr]            # (18, 64, 3, 3)
    b_off_r = b_off[idxr]            # (18,)
    woff = np.ascontiguousarray(
        W_off_r.transpose(2, 3, 1, 0).reshape(9, 64, 18).transpose(1, 0, 2)
    ).reshape(64, 9 * 18).astype(np.float32)
    # base2 [128 w, 64 t, 18]
    nidx = np.arange(9)
    pnx = (nidx // 3) - 1
    pny = (nidx % 3) - 1
    tt = np.arange(64)
    ww = np.arange(128)
    base2 = np.zeros((128, 64, 18), np.float32)
    base2[:, :, 0:9] = tt[None, :, None] + 9 + pnx[None, None, :] + \
        b_off_r[None, None, 0:9]
    base2[:, :, 9:18] = ww[:, None, None] + 9 + pny[None, None, :] + \
        b_off_r[None, None, 9:18]
    base2 = base2.reshape(128, 64 * 18)
    # final conv weights
    Wmat = W_conv.reshape(64, 64, 9).transpose(0, 2, 1)   # (co, n, ci)
    wca = np.zeros((128, 256), np.float32)
    for jc in range(4):
        for dn in range(2):
            wca[dn * 64:(dn + 1) * 64, jc * 64:(jc + 1) * 64] = \
                Wmat[:, 2 * jc + dn, :].T
    wcb = np.ascontiguousarray(Wmat[:, 8, :].T)           # (ci, co)
    # K-packed offset-conv weights: woffa[(j,ci), dy, :] = woff tap (dy, j)
    woff3 = woff.reshape(64, 9, 18)
    woffa = np.zeros((128, 3, 18), np.float32)
    for dy in range(3):
        woffa[0:64, dy] = woff3[:, dy * 3 + 0]
        woffa[64:128, dy] = woff3[:, dy * 3 + 1]
    return {
        "woff": woff,
        "woffa": woffa.reshape(128, 54),
        "base2": base2,
        "wconv_a": wca.astype(BF16),
        "wconv_b": wcb.astype(BF16),
        "ident_f": np.eye(128, dtype=np.float32),
        "ident_b": np.eye(128, dtype=np.float32).astype(BF16),
    }


def _host_xd(xgs):
    """Column-major corner-pair gather source. slot (c, k) holds
    [x(k, c)(64ch), x(k, c+1)(64ch)] bf16; slot index = c*NRC + k."""
    xz = np.zeros((64, NRC, NC2 + 1), np.float32)
    xz[:, :NR, 1:129] = xgs
    T = np.ascontiguousarray(xz.transpose(2, 1, 0))     # [131c, 76k, 64ch]
    pair = np.concatenate([T[0:NC2], T[1:NC2 + 1]], axis=2)  # [130, 76, 128]
    xdarr = np.zeros((XD2T, 128), BF16)
    xdarr[:NSLOT2] = pair.reshape(NSLOT2, 128).astype(BF16)
    return xdarr


def _per_core_inputs(x, consts, s, half):
    h0 = 64 * half
    xs = x[s]                                    # (64, 128, 128)
    xgs = np.zeros((64, NR, 128), np.float32)
    lo = h0 - 5                                  # unpadded row of xg row 0
    for k in range(NR):
        r = lo + k
        if 0 <= r < 128:
            xgs[:, k, :] = xs[:, r, :]
    xsc = np.zeros((128, 4), np.float32)
    xsc[:, 0] = 9 - h0                           # mask lo
    xsc[:, 1] = 136 - h0                         # mask hi
    xsc[:, 2] = 8 - min(h0, 2)                   # clip lo (tightened)
    xsc[:, 3] = min(129, h0 + 69) - h0 + 8       # clip hi (tightened)
    return {
        "xg": xgs.reshape(64, NR * 128),
        "xd": _host_xd(xgs),
        "xsc": xsc,
        **consts,
    }


def kernel(x, W_off, b_off, W_conv):
    _install_ntff_hook()
    # the bass kernel must run on the axon trn2 backend; undo any cpu pin
    # (e.g. a harness that set JAX_PLATFORMS=cpu for the reference)
    import os
    if os.environ.get("JAX_PLATFORMS", "") == "cpu":
        try:
            import jax
            jax.config.update("jax_platforms", None)
            os.environ.pop("JAX_PLATFORMS", None)
        except Exception:
            pass
    x = np.asarray(x, np.float32)
    W_off = np.asarray(W_off, np.float32)
    b_off = np.asarray(b_off, np.float32)
    W_conv = np.asarray(W_conv, np.float32)

    from concourse.bass_utils import run_bass_kernel_spmd
    nc = _build_program()
    consts = _host_consts(W_off, b_off, W_conv)
    in_maps = [
        _per_core_inputs(x, consts, core // 2, core % 2) for core in range(NCORES)
    ]
    res = run_bass_kernel_spmd(nc, in_maps, list(range(NCORES)))
    out = np.empty((4, 64, 128, 128), np.float32)
    for core in range(NCORES):
        s, half = core // 2, core % 2
        out[s, :, 64 * half:64 * half + 64, :] = \
            res.results[core]["out"].reshape(64, 64, 128)
    return out
